# revision 49
# baseline (speedup 1.0000x reference)
"""Trainium2 Bass kernel for nn_AwesomeGRU (SEQ=512, B=64, DIM=1024, UNITS=1024).

Algorithm: the `reset` input zeroes h *before* each masked step, so each batch
row's recurrence splits into independent segments (h carries over only within
a segment). Classic packed-sequence reformulation:

  host: enumerate segments, sort by length desc, deal round-robin to 8 cores,
        lay tokens out depth-major ((depth, segment-rank) order). Pass j
        processes all tokens at depth j — a contiguous row block whose h
        inputs are a PREFIX of pass j-1's outputs (no gather).
  core: for each pass j: PSUM <- x_j @ W_ih^T (+ h_j @ W_hh^T if j>0), then
        gates elementwise, h_out -> DRAM (it IS the output) + fp16 copy in
        SBUF for pass j+1's matmul.
  host: inverse-permute output tokens to (seq, b, units).

Everything is feature-major on device: activations stored (units, rows) so
no transposes are ever needed. Matmul operands fp16 (same PE rate as bf16 on
TRN2, 3 more mantissa bits; PSUM accumulates fp32), elementwise fp32. Depth-0
tokens (h=0) skip the h-matmul exactly.

Self-contained: derives everything from the runtime value of `reset`.
"""
import os
import numpy as np

import concourse.bacc as bacc
import concourse.mybir as mybir
import concourse.tile as tile
from concourse.bass_utils import run_bass_kernel_spmd

SEQ, B, DIM, UNITS = 512, 64, 1024, 1024
NCORES = 8
P = 128
CG = DIM // P        # 8 contraction groups per matmul side
UG = UNITS // P      # 8 unit groups
CH = 512             # row-chunk (free dim / PSUM bank)
FUSE_M = 64          # passes with m <= FUSE_M pack all UG unit tiles per bank
dt = mybir.dt
f32 = dt.float32
bf16 = dt.float16  # fp16: same PE rate as bf16, 3 more mantissa bits

LAST_EXEC_NS = None  # set when GRU_TRACE=1
LAST_RES = None


# ---------------------------------------------------------------- host plan

def _build_plan(reset_sb, h0_any):
    """reset_sb: (SEQ, B) bool. Returns (m_j schedule, per-core token maps).

    Segment starts: t=0 always (h0 seed row: h0[b] unless reset[0,b]), and
    every t>0 with reset=1 (h zeroed exactly).
    """
    segs = []  # (length, b, t_start)
    for b in range(B):
        col = reset_sb[:, b]
        starts = [0] + [t for t in range(1, SEQ) if col[t]]
        for i, s in enumerate(starts):
            e = starts[i + 1] if i + 1 < len(starts) else SEQ
            segs.append((e - s, b, s))
    segs.sort(key=lambda x: (-x[0], x[1], x[2]))
    Lmax = segs[0][0]
    n_j = [0] * Lmax
    for L, _, _ in segs:
        for j in range(L):
            n_j[j] += 1
    m_j = [(n + NCORES - 1) // NCORES for n in n_j]

    plans = []
    for c in range(NCORES):
        mysegs = segs[c::NCORES]
        tok = np.full(sum(m_j), -1, np.int64)  # flat t*B+b index or -1 pad
        seed_b = np.full(m_j[0], -1, np.int64)  # batch row for h seed (pass 0)
        off = 0
        for j in range(Lmax):
            for r in range(m_j[j]):
                if r < len(mysegs) and mysegs[r][0] > j:
                    L, b, s = mysegs[r]
                    tok[off + r] = (s + j) * B + b
                    if j == 0 and s == 0 and h0_any and not reset_sb[0, b]:
                        seed_b[r] = b
            off += m_j[j]
        plans.append((tok, seed_b))
    return m_j, plans


# ------------------------------------------------------------- device build

def _pick_j_fuse(m_j, j_pre):
    """First pass run in fused mode: fits 2 row-chunks of <= FUSE_M, and the
    next pass's h rows must come from the first chunk alone."""
    L = len(m_j)
    j = max(j_pre, 1)
    while j < L and not (m_j[j] <= 2 * FUSE_M
                         and (j + 1 >= L or m_j[j + 1] <= FUSE_M)):
        j += 1
    return j


def _chunks(m):
    """Split m rows into balanced chunks of <= CH."""
    nch = (m + CH - 1) // CH
    base, rem = divmod(m, nch)
    out, off = [], 0
    for i in range(nch):
        f = base + (1 if i < rem else 0)
        out.append((off, f))
        off += f
    return out


def _build_nc(m_j, use_seed, j_pre):
    """j_pre: first pass whose gi comes from the fp16 presweep buffer."""
    Lmax = len(m_j)
    N_pad = sum(m_j)
    M_off = np.cumsum([0] + m_j)  # row offset of each pass block
    R0 = int(M_off[j_pre]) if j_pre < Lmax else N_pad  # presweep row range
    RN = N_pad - R0
    # First fused pass: all UG unit tiles share one PSUM bank per gate.
    j_fuse = _pick_j_fuse(m_j, j_pre)
    R5 = int(M_off[j_fuse]) if j_fuse < Lmax else N_pad
    TAILN = N_pad - R5  # fused-tail rows, buffered in SBUF until the end

    nc = bacc.Bacc("TRN2", target_bir_lowering=False, debug=False,
                   num_devices=NCORES)
    # p-major 3D layouts: [p, c, col] so one strided DMA fills a whole slab
    xT = nc.dram_tensor("xT", [P, CG, N_pad], bf16, kind="ExternalInput")
    wihT = nc.dram_tensor("wihT", [P, CG, 3 * UNITS], bf16,
                          kind="ExternalInput")
    whhT = nc.dram_tensor("whhT", [P, CG, 3 * UNITS], bf16,
                          kind="ExternalInput")
    biases = nc.dram_tensor("biases", [P, UG, 4], f32, kind="ExternalInput")
    outT = nc.dram_tensor("outT", [UNITS, N_pad], f32, kind="ExternalOutput")
    outTail = (nc.dram_tensor("outTail", [P, UG, TAILN], bf16,
                              kind="ExternalOutput") if TAILN > 0 else None)
    bhhn_bc = (nc.dram_tensor("bhhn_bc", [P, UG, FUSE_M], bf16,
                              kind="ExternalInput") if TAILN > 0 else None)
    hseedT = None
    if use_seed:
        hseedT = nc.dram_tensor("hseedT", [UNITS, m_j[0]], bf16,
                                kind="ExternalInput")

    Sig = mybir.ActivationFunctionType.Sigmoid
    Tanh = mybir.ActivationFunctionType.Tanh
    ADD = mybir.AluOpType.add
    MULT = mybir.AluOpType.mult

    with tile.TileContext(nc) as tc:
        with (
            tc.tile_pool(name="wpool", bufs=1) as wpool,
            tc.tile_pool(name="xpool", bufs=2) as xpool,
            tc.tile_pool(name="hpool", bufs=2) as hpool,
            tc.tile_pool(name="spool", bufs=2) as spool,
            tc.tile_pool(name="ppool", bufs=2, space="PSUM") as ppool,
        ):
            wih_t = wpool.tile([P, CG, 3 * UNITS], bf16, tag="wih")
            whh_t = wpool.tile([P, CG, 3 * UNITS], bf16, tag="whh")

            x_tiles = {}

            def get_x_tile(jj, ooff, ff, step=2):
                key = (jj, ooff)
                if key not in x_tiles:
                    x_t = xpool.tile([P, CG, CH], bf16, tag="x", name="x_t")
                    bb = int(M_off[jj]) + ooff
                    for cp in range(0, CG, step):
                        nc.sync.dma_start(out=x_t[:, cp:cp + step, :ff],
                                          in_=xT[:, cp:cp + step, bb: bb + ff])
                    x_tiles[key] = x_t
                return x_tiles[key]

            def wih_slab(g, u0, u1):
                nc.sync.dma_start(
                    out=wih_t[:, :, g * UNITS + u0 * P: g * UNITS + u1 * P],
                    in_=wihT[:, :, g * UNITS + u0 * P: g * UNITS + u1 * P])

            # DMA emission order = consumption order: u0's r-gate slab and the
            # first x chunk unblock the very first matmul group; then the
            # other u0 gates, biases, the rest of W_ih, the second x chunk.
            # W_hh and the presweep are emitted later.
            ch0 = _chunks(m_j[0])
            wih_slab(0, 0, 1)
            get_x_tile(0, *ch0[0])
            wih_slab(1, 0, 1)
            wih_slab(2, 0, 1)
            b_t = wpool.tile([P, UG, 4], f32, tag="bias")
            nc.sync.dma_start(out=b_t[:, :, :], in_=biases[:, :, :])
            for g in range(3):
                wih_slab(g, 1, 2)
            for g in range(3):
                wih_slab(g, 2, 8)
            if len(ch0) > 1:
                get_x_tile(0, *ch0[1])

            def emit_whh():
                for g in range(3):
                    nc.sync.dma_start(
                        out=whh_t[:, :, g * UNITS:(g + 1) * UNITS],
                        in_=whhT[:, :, g * UNITS:(g + 1) * UNITS])

            gi_pre = (wpool.tile([P, 3 * UG, RN], dt.float16, tag="gi_pre",
                                name="gi_pre")
                      if RN > 0 else None)
            Copy = mybir.ActivationFunctionType.Identity  # Copy forbids AP bias

            def emit_presweep():
                # gi for all deep-pass rows in one efficient batched matmul.
                # Gate biases (b_sum_r, b_sum_z, b_ihn) are folded in here via
                # the activation bias, so pre-pass gates add gi_pre directly.
                with nc.named_scope("presweep"):
                    xp_t = xpool.tile([P, CG, RN], bf16, tag="x", name="xp_t")
                    for cp in range(0, CG, 2):
                        nc.sync.dma_start(out=xp_t[:, cp:cp + 2, :],
                                          in_=xT[:, cp:cp + 2, R0:N_pad])
                    for gu in range(3 * UG):
                        g, u = divmod(gu, UG)
                        ps_p = ppool.tile([P, CH], f32, tag="ps_gin",
                                          name="ps_pre")
                        for c in range(CG):
                            nc.tensor.matmul(
                                ps_p[:, :RN],
                                lhsT=wih_t[:, c, gu * P:(gu + 1) * P],
                                rhs=xp_t[:, c, :],
                                start=(c == 0), stop=(c == CG - 1))
                        nc.scalar.activation(gi_pre[:, gu, :], ps_p[:, :RN],
                                             Copy, bias=b_t[:, u, g:g + 1])

            if use_seed:
                emit_whh()  # pass 0 already needs W_hh

            # Fused-tail state: one PSUM bank per gate holds all UG unit
            # tiles (packed by m); elementwise runs as UG-wide ops; outputs
            # buffer in SBUF and ship in one DMA at the end.
            fused_state = {}

            f16 = dt.float16

            def emit_fused_chunk(j, h_cur, off, m, h_next, h_next_m=0):
                ob = fused_state["ob"]
                bb = fused_state["bb"]
                base = int(M_off[j])
                p0 = base + off - R0
                t0 = base + off - R5
                ps_z = ppool.tile([P, UG, FUSE_M], f32, tag="ps_z")
                ps_r = ppool.tile([P, UG, FUSE_M], f32, tag="ps_r")
                ps_n = ppool.tile([P, UG, FUSE_M], f32, tag="ps_ghn",
                                  name="ps_n")

                def gate_mms(gate, ps):
                    for u in range(UG):
                        for c in range(CG):
                            nc.tensor.matmul(
                                ps[:, u, :m],
                                lhsT=whh_t[:, c, gate * UNITS + u * P:
                                           gate * UNITS + (u + 1) * P],
                                rhs=h_cur[:, c, off:off + m],
                                start=(c == 0),
                                stop=(c == CG - 1),
                                skip_group_check=True)

                def gi3(g):
                    return gi_pre[:, g * UG:(g + 1) * UG, p0:p0 + m]

                def stile(tag):
                    return spool.tile([P, UG, FUSE_M], f16, tag=tag, name=tag,
                                      bufs=1)

                # z first: its whole chain hides under the r/ghn matmuls
                gate_mms(1, ps_z)
                z_sb = stile("fz")
                nc.vector.tensor_add(z_sb[:, :, :m], ps_z[:, :, :m], gi3(1))
                nc.scalar.activation(z_sb[:, :, :m], z_sb[:, :, :m], Sig)
                zh = stile("fzh")
                nc.vector.tensor_mul(zh[:, :, :m], z_sb[:, :, :m],
                                     h_cur[:, :, off:off + m])
                zm = stile("fzm")  # 1 - z
                nc.scalar.activation(zm[:, :, :m], z_sb[:, :, :m], Copy,
                                     bias=1.0, scale=-1.0)
                gate_mms(0, ps_r)
                r_sb = stile("fr")
                nc.vector.tensor_add(r_sb[:, :, :m], ps_r[:, :, :m], gi3(0))
                nc.scalar.activation(r_sb[:, :, :m], r_sb[:, :, :m], Sig)
                # n = tanh(gi_n + r*b_hhn + r*gh_n): r*b_hhn + gi_n computed
                # while the ghn matmuls stream, so the tail chain stays short
                rb = stile("ft")
                nc.vector.tensor_mul(rb[:, :, :m], r_sb[:, :, :m],
                                     bb[:, :, :m])
                grb = stile("fg")
                nc.vector.tensor_add(grb[:, :, :m], rb[:, :, :m], gi3(2))
                gate_mms(2, ps_n)
                g_sb = stile("ft")
                nc.vector.tensor_mul(g_sb[:, :, :m], ps_n[:, :, :m],
                                     r_sb[:, :, :m])
                n_sb = stile("fn")
                nc.vector.tensor_add(n_sb[:, :, :m], g_sb[:, :, :m],
                                     grb[:, :, :m])
                nc.scalar.activation(n_sb[:, :, :m], n_sb[:, :, :m], Tanh)
                a_sb = stile("fg")
                nc.vector.tensor_mul(a_sb[:, :, :m], zm[:, :, :m],
                                     n_sb[:, :, :m])
                if h_next is not None:
                    # split by c-halves: the next pass's first matmuls (c<4)
                    # unblock after the first half lands
                    nc.vector.tensor_add(h_next[:, 0:4, :],
                                         a_sb[:, 0:4, :h_next_m],
                                         zh[:, 0:4, :h_next_m])
                    nc.vector.tensor_add(h_next[:, 4:8, :],
                                         a_sb[:, 4:8, :h_next_m],
                                         zh[:, 4:8, :h_next_m])
                nc.vector.tensor_add(ob[:, :, t0:t0 + m], a_sb[:, :, :m],
                                     zh[:, :, :m])

            def emit_fused_pass(j, h_cur):
                if not fused_state:
                    ob = wpool.tile([P, UG, TAILN], f16, tag="outbuf")
                    bb = wpool.tile([P, UG, FUSE_M], f16, tag="bb")
                    nc.sync.dma_start(out=bb[:, :, :], in_=bhhn_bc[:, :, :])
                    fused_state.update(ob=ob, bb=bb)
                m = m_j[j]
                m_next = m_j[j + 1] if j + 1 < Lmax else 0
                h_next = (hpool.tile([P, CG, m_next], bf16, tag="hbuf",
                                     name=f"hbuf{j}") if m_next > 0 else None)
                # chunk 0 first: it alone carries h for the next pass, so its
                # chain (and h_next) completes while later chunks still stream
                for off in range(0, m, FUSE_M):
                    mc = min(FUSE_M, m - off)
                    emit_fused_chunk(j, h_cur, off, mc,
                                     h_next if off == 0 else None, m_next)
                if j == Lmax - 1:
                    # one fat-descriptor DMA for the whole tail (p-major)
                    nc.sync.dma_start(out=outTail[:, :, :],
                                      in_=fused_state["ob"][:, :, :])
                return h_next

            h_cur = None  # bf16 SBUF (P, CG, m_j[j]) input h for current pass
            for j in range(Lmax):
                if j == j_pre and gi_pre is not None:
                    emit_presweep()
                scope = nc.named_scope(f"pass{j:02d}")
                scope.__enter__()
                if j >= j_fuse:
                    h_cur = emit_fused_pass(j, h_cur)
                    scope.__exit__(None, None, None)
                    continue
                m = m_j[j]
                m_next = m_j[j + 1] if j + 1 < Lmax else 0
                has_h = (j > 0) or use_seed
                pre = j >= j_pre
                base = int(M_off[j])
                h_next = (hpool.tile([P, CG, m_next], bf16, tag="hbuf",
                                     name=f"hbuf{j}")
                          if m_next > 0 else None)

                for ci, (off, f) in enumerate(_chunks(m)):
                    if not pre:
                        x_t = get_x_tile(j, off, f)
                    if j == 0 and use_seed:
                        hs_t = xpool.tile([P, CG, CH], bf16, tag="hseed", name="hs_t", bufs=1)
                        for c in range(CG):
                            nc.sync.dma_start(
                                out=hs_t[:, c, :f],
                                in_=hseedT[c * P:(c + 1) * P, off: off + f])
                        h_in = lambda c: hs_t[:, c, :f]
                    elif has_h:
                        h_in = lambda c: h_cur[:, c, off: off + f]
                    else:
                        h_in = None
                    # presweep-relative row slice for this chunk
                    p0 = base + off - R0

                    def x_mms(ps, gate, stop_at_end):
                        for c in range(CG):
                            nc.tensor.matmul(
                                ps[:, :f],
                                lhsT=wih_t[:, c, gate * UNITS + u * P:
                                           gate * UNITS + (u + 1) * P],
                                rhs=x_t[:, c, :f],
                                start=(c == 0),
                                stop=(stop_at_end and c == CG - 1))

                    def h_mms(ps, gate, cs, do_start, do_stop):
                        cs = list(cs)
                        for c in cs:
                            nc.tensor.matmul(
                                ps[:, :f],
                                lhsT=whh_t[:, c, gate * UNITS + u * P:
                                           gate * UNITS + (u + 1) * P],
                                rhs=h_in(c),
                                start=(do_start and c == cs[0]),
                                stop=(do_stop and c == cs[-1]),
                                skip_group_check=True)

                    for u in range(UG):
                        ps_r = ppool.tile([P, CH], f32, tag="ps_r")
                        ps_z = ppool.tile([P, CH], f32, tag="ps_z")
                        if not pre:
                            ps_gin = ppool.tile([P, CH], f32, tag="ps_gin")
                        ps_ghn = (ppool.tile([P, CH], f32, tag="ps_ghn",
                                             name="ps_ghn")
                                  if has_h else None)

                        # For the first unit-tile of a chunk, defer every
                        # gate's c=7 h-matmul to the end: it waits on the
                        # previous pass's last h cast, and deferring lets the
                        # other 21+ matmuls run during that wait.
                        split = has_h and u == 0 and off == 0
                        early = range(CG - 1) if split else range(CG)
                        if not pre:
                            x_mms(ps_r, 0, stop_at_end=not has_h)
                            if has_h:
                                h_mms(ps_r, 0, early, False, not split)
                            x_mms(ps_z, 1, stop_at_end=not has_h)
                            if has_h:
                                h_mms(ps_z, 1, early, False, not split)
                            x_mms(ps_gin, 2, stop_at_end=True)
                            if has_h:
                                h_mms(ps_ghn, 2, early, True, not split)
                        else:
                            h_mms(ps_r, 0, early, True, not split)
                            h_mms(ps_z, 1, early, True, not split)
                            h_mms(ps_ghn, 2, early, True, not split)
                        if split:
                            h_mms(ps_r, 0, [CG - 1], False, True)
                            h_mms(ps_z, 1, [CG - 1], False, True)
                            h_mms(ps_ghn, 2, [CG - 1], False, True)

                        r_sb = spool.tile([P, CH], f32, tag="r")
                        z_sb = spool.tile([P, CH], f32, tag="z")
                        n_sb = spool.tile([P, CH], f32, tag="n")
                        h_sb = spool.tile([P, CH], f32, tag="r" if use_seed else "h",
                                          name="h_sb")
                        t2 = spool.tile([P, CH], f32, tag="t2")
                        if pre:
                            # biases already folded into gi_pre at presweep
                            nc.vector.tensor_add(r_sb[:, :f], ps_r[:, :f],
                                                 gi_pre[:, u, p0:p0 + f])
                            nc.scalar.activation(r_sb[:, :f], r_sb[:, :f], Sig)
                            nc.vector.tensor_add(z_sb[:, :f], ps_z[:, :f],
                                                 gi_pre[:, UG + u, p0:p0 + f])
                            nc.scalar.activation(z_sb[:, :f], z_sb[:, :f], Sig)
                            nc.vector.scalar_tensor_tensor(
                                t2[:, :f], ps_ghn[:, :f], b_t[:, u, 3:4],
                                r_sb[:, :f], op0=ADD, op1=MULT)
                            arg = spool.tile([P, CH], f32, tag="d", name="arg")
                            nc.vector.tensor_add(arg[:, :f], t2[:, :f],
                                                 gi_pre[:, 2 * UG + u, p0:p0 + f])
                            nc.scalar.activation(n_sb[:, :f], arg[:, :f], Tanh)
                        else:
                            nc.scalar.activation(r_sb[:, :f], ps_r[:, :f], Sig,
                                                 bias=b_t[:, u, 0:1])
                            nc.scalar.activation(z_sb[:, :f], ps_z[:, :f], Sig,
                                                 bias=b_t[:, u, 1:2])
                            if has_h:
                                # t2 = (ps_ghn + b_hhn) * r
                                nc.vector.scalar_tensor_tensor(
                                    t2[:, :f], ps_ghn[:, :f], b_t[:, u, 3:4],
                                    r_sb[:, :f], op0=ADD, op1=MULT)
                                arg = spool.tile([P, CH], f32, tag="d", name="arg")
                                nc.vector.tensor_add(arg[:, :f], t2[:, :f],
                                                     ps_gin[:, :f])
                                nc.scalar.activation(n_sb[:, :f], arg[:, :f],
                                                     Tanh, bias=b_t[:, u, 2:3])
                            else:
                                # t2 = r*b_hhn + ps_gin ; n = tanh(t2 + b_ihn)
                                nc.vector.scalar_tensor_tensor(
                                    t2[:, :f], r_sb[:, :f], b_t[:, u, 3:4],
                                    ps_gin[:, :f], op0=MULT, op1=ADD)
                                nc.scalar.activation(n_sb[:, :f], t2[:, :f],
                                                     Tanh, bias=b_t[:, u, 2:3])
                        if has_h:
                            # h = n + z*(h_prev - n)   (h_prev via bf16 tile)
                            d_sb = spool.tile([P, CH], f32, tag="d")
                            nc.vector.tensor_sub(d_sb[:, :f], h_in(u), n_sb[:, :f])
                            zd = spool.tile([P, CH], f32, tag="t2", name="zd")
                            nc.vector.tensor_mul(zd[:, :f], z_sb[:, :f], d_sb[:, :f])
                            nc.vector.tensor_add(h_sb[:, :f], n_sb[:, :f], zd[:, :f])
                        else:
                            # h = (1-z)*n = n - z*n
                            zd = spool.tile([P, CH], f32, tag="t2", name="zd")
                            nc.vector.tensor_mul(zd[:, :f], z_sb[:, :f], n_sb[:, :f])
                            nc.vector.tensor_sub(h_sb[:, :f], n_sb[:, :f], zd[:, :f])

                        nc.sync.dma_start(
                            out=outT[u * P:(u + 1) * P, base + off: base + off + f],
                            in_=h_sb[:, :f])
                        pf = min(m_next - off, f)
                        if pf > 0:
                            nc.vector.tensor_copy(h_next[:, u, off: off + pf],
                                                  h_sb[:, :pf])
                    if j == 0 and ci == 0 and not use_seed:
                        emit_whh()  # W_hh drains during pass-0 compute
                    if not pre and (j, off) in x_tiles:
                        del x_tiles[(j, off)]  # consumed; let the slot recycle
                h_cur = h_next
                scope.__exit__(None, None, None)
    nc.compile()
    return nc


# ------------------------------------------------------------------- kernel

def kernel(x, h0, reset, W_ih, W_hh, b_ih, b_hh):
    global LAST_EXEC_NS
    x = np.asarray(x, np.float32)
    h0 = np.asarray(h0, np.float32)
    reset_sb = np.asarray(reset).reshape(SEQ, B).astype(bool)
    W_ih = np.asarray(W_ih, np.float32)
    W_hh = np.asarray(W_hh, np.float32)
    b_ih = np.asarray(b_ih, np.float32)
    b_hh = np.asarray(b_hh, np.float32)

    h0_any = bool(np.any(h0))
    m_j, plans = _build_plan(reset_sb, h0_any)
    N_pad = sum(m_j)

    b_sum = b_ih + b_hh
    biases = np.stack([b_sum[:UNITS], b_sum[UNITS:2 * UNITS],
                       b_ih[2 * UNITS:], b_hh[2 * UNITS:]], axis=1)
    # p-major 3D layouts matching the device DMA access patterns
    biases3 = np.ascontiguousarray(
        biases.reshape(UG, P, 4).transpose(1, 0, 2), np.float32)
    wih3 = np.ascontiguousarray(
        W_ih.reshape(3 * UNITS, CG, P).transpose(2, 1, 0)).astype(np.float16)
    whh3 = np.ascontiguousarray(
        W_hh.reshape(3 * UNITS, CG, P).transpose(2, 1, 0)).astype(np.float16)
    bhhn = b_hh[2 * UNITS:].reshape(UG, P)  # (u, p)
    bhhn_bc = np.ascontiguousarray(
        np.broadcast_to(bhhn.T[:, :, None], (P, UG, FUSE_M))).astype(np.float16)

    j_pre = 1
    while j_pre < len(m_j) and sum(m_j[j_pre:]) > CH:
        j_pre += 1
    j_fuse = _pick_j_fuse(m_j, j_pre)
    has_fused = j_fuse < len(m_j)

    xf = x.reshape(SEQ * B, DIM)
    in_maps = []
    for c in range(NCORES):
        tok, seed_b = plans[c]
        real = tok >= 0
        xg = np.zeros((N_pad, DIM), np.float32)
        xg[real] = xf[tok[real]]
        m = {
            "xT": np.ascontiguousarray(
                xg.reshape(N_pad, CG, P).transpose(2, 1, 0)).astype(np.float16),
            "wihT": wih3, "whhT": whh3, "biases": biases3,
        }
        if has_fused:
            m["bhhn_bc"] = bhhn_bc
        if h0_any:
            hs = np.zeros((m_j[0], UNITS), np.float32)
            sreal = seed_b >= 0
            hs[sreal] = h0[seed_b[sreal]]
            m["hseedT"] = np.ascontiguousarray(hs.T).astype(np.float16)
        in_maps.append(m)

    nc = _build_nc(m_j, use_seed=h0_any, j_pre=j_pre)
    trace = os.environ.get("GRU_TRACE", "0") == "1"
    res = run_bass_kernel_spmd(nc, in_maps, list(range(NCORES)), trace=trace)
    global LAST_RES
    LAST_RES = res
    LAST_EXEC_NS = res.exec_time_ns

    R5 = int(sum(m_j[:j_fuse]))
    out = np.zeros((SEQ * B, UNITS), np.float32)
    for c in range(NCORES):
        tok, _ = plans[c]
        real = tok >= 0
        full = res.results[c]["outT"].T.copy()  # (N_pad, UNITS)
        if has_fused:
            tail = res.results[c]["outTail"]  # (P, UG, TAILN)
            full[R5:] = tail.transpose(2, 1, 0).reshape(
                N_pad - R5, UNITS).astype(np.float32)
        out[tok[real]] = full[real]
    return out.reshape(SEQ, B, UNITS)



# revision 50
# speedup vs baseline: 1.1901x; 1.1901x over previous
"""Trainium2 Bass kernel for nn_AwesomeGRU (SEQ=512, B=64, DIM=1024, UNITS=1024).

Algorithm: the `reset` input zeroes h *before* each masked step, so each batch
row's recurrence splits into independent segments (h carries over only within
a segment). Classic packed-sequence reformulation:

  host: enumerate segments, sort by length desc, deal round-robin to 8 cores,
        lay tokens out depth-major ((depth, segment-rank) order). Pass j
        processes all tokens at depth j — a contiguous row block whose h
        inputs are a PREFIX of pass j-1's outputs (no gather).
  core: for each pass j: PSUM <- x_j @ W_ih^T (+ h_j @ W_hh^T if j>0), then
        gates elementwise, h_out -> DRAM (it IS the output) + fp16 copy in
        SBUF for pass j+1's matmul.
  host: inverse-permute output tokens to (seq, b, units).

Everything is feature-major on device: activations stored (units, rows) so
no transposes are ever needed. Matmul operands fp16 (same PE rate as bf16 on
TRN2, 3 more mantissa bits; PSUM accumulates fp32), elementwise fp32. Depth-0
tokens (h=0) skip the h-matmul exactly.

Self-contained: derives everything from the runtime value of `reset`.
"""
import os
import numpy as np

import concourse.bacc as bacc
import concourse.mybir as mybir
import concourse.tile as tile
from concourse.bass_utils import run_bass_kernel_spmd

SEQ, B, DIM, UNITS = 512, 64, 1024, 1024
NCORES = 8
P = 128
CG = DIM // P        # 8 contraction groups per matmul side
UG = UNITS // P      # 8 unit groups
CH = 512             # row-chunk (free dim / PSUM bank)
FUSE_M = 64          # passes with m <= FUSE_M pack all UG unit tiles per bank
dt = mybir.dt
f32 = dt.float32
bf16 = dt.float16  # fp16: same PE rate as bf16, 3 more mantissa bits

LAST_EXEC_NS = None  # set when GRU_TRACE=1
LAST_RES = None


# ---------------------------------------------------------------- host plan

def _build_plan(reset_sb, h0_any):
    """reset_sb: (SEQ, B) bool. Returns (m_j schedule, per-core token maps).

    Segment starts: t=0 always (h0 seed row: h0[b] unless reset[0,b]), and
    every t>0 with reset=1 (h zeroed exactly).
    """
    segs = []  # (length, b, t_start)
    for b in range(B):
        col = reset_sb[:, b]
        starts = [0] + [t for t in range(1, SEQ) if col[t]]
        for i, s in enumerate(starts):
            e = starts[i + 1] if i + 1 < len(starts) else SEQ
            segs.append((e - s, b, s))
    segs.sort(key=lambda x: (-x[0], x[1], x[2]))
    Lmax = segs[0][0]
    n_j = [0] * Lmax
    for L, _, _ in segs:
        for j in range(L):
            n_j[j] += 1
    m_j = [(n + NCORES - 1) // NCORES for n in n_j]

    plans = []
    for c in range(NCORES):
        mysegs = segs[c::NCORES]
        tok = np.full(sum(m_j), -1, np.int64)  # flat t*B+b index or -1 pad
        seed_b = np.full(m_j[0], -1, np.int64)  # batch row for h seed (pass 0)
        off = 0
        for j in range(Lmax):
            for r in range(m_j[j]):
                if r < len(mysegs) and mysegs[r][0] > j:
                    L, b, s = mysegs[r]
                    tok[off + r] = (s + j) * B + b
                    if j == 0 and s == 0 and h0_any and not reset_sb[0, b]:
                        seed_b[r] = b
            off += m_j[j]
        plans.append((tok, seed_b))
    return m_j, plans


# ------------------------------------------------------------- device build

def _pick_j_fuse(m_j, j_pre):
    """First pass run in fused mode: fits 2 row-chunks of <= FUSE_M, and the
    next pass's h rows must come from the first chunk alone."""
    L = len(m_j)
    j = max(j_pre, 1)
    while j < L and not (m_j[j] <= 2 * FUSE_M
                         and (j + 1 >= L or m_j[j + 1] <= FUSE_M)):
        j += 1
    return j


def _chunks(m):
    """Split m rows into balanced chunks of <= CH."""
    nch = (m + CH - 1) // CH
    base, rem = divmod(m, nch)
    out, off = [], 0
    for i in range(nch):
        f = base + (1 if i < rem else 0)
        out.append((off, f))
        off += f
    return out


def _build_nc(m_j, use_seed, j_pre):
    """j_pre: first pass whose gi comes from the fp16 presweep buffer."""
    Lmax = len(m_j)
    N_pad = sum(m_j)
    M_off = np.cumsum([0] + m_j)  # row offset of each pass block
    R0 = int(M_off[j_pre]) if j_pre < Lmax else N_pad  # presweep row range
    RN = N_pad - R0
    # First fused pass: all UG unit tiles share one PSUM bank per gate.
    j_fuse = _pick_j_fuse(m_j, j_pre)
    R5 = int(M_off[j_fuse]) if j_fuse < Lmax else N_pad
    TAILN = N_pad - R5  # fused-tail rows, buffered in SBUF until the end

    nc = bacc.Bacc("TRN2", target_bir_lowering=False, debug=False,
                   num_devices=NCORES)
    # p-major 3D layouts: [p, c, col] so one strided DMA fills a whole slab
    xT = nc.dram_tensor("xT", [P, CG, N_pad], bf16, kind="ExternalInput")
    wihT = nc.dram_tensor("wihT", [P, CG, 3 * UNITS], bf16,
                          kind="ExternalInput")
    whhT = nc.dram_tensor("whhT", [P, CG, 3 * UNITS], bf16,
                          kind="ExternalInput")
    biases = nc.dram_tensor("biases", [P, UG, 4], f32, kind="ExternalInput")
    outT = nc.dram_tensor("outT", [UNITS, N_pad], f32, kind="ExternalOutput")
    outTail = (nc.dram_tensor("outTail", [P, UG, TAILN], bf16,
                              kind="ExternalOutput") if TAILN > 0 else None)
    bhhn_bc = (nc.dram_tensor("bhhn_bc", [P, UG, FUSE_M], bf16,
                              kind="ExternalInput") if TAILN > 0 else None)
    hseedT = None
    if use_seed:
        hseedT = nc.dram_tensor("hseedT", [UNITS, m_j[0]], bf16,
                                kind="ExternalInput")

    Sig = mybir.ActivationFunctionType.Sigmoid
    Tanh = mybir.ActivationFunctionType.Tanh
    ADD = mybir.AluOpType.add
    MULT = mybir.AluOpType.mult

    with tile.TileContext(nc) as tc:
        with (
            tc.tile_pool(name="wpool", bufs=1) as wpool,
            tc.tile_pool(name="xpool", bufs=2) as xpool,
            tc.tile_pool(name="hpool", bufs=2) as hpool,
            tc.tile_pool(name="spool", bufs=2) as spool,
            tc.tile_pool(name="ppool", bufs=2, space="PSUM") as ppool,
        ):
            wih_t = wpool.tile([P, CG, 3 * UNITS], bf16, tag="wih")
            whh_t = wpool.tile([P, CG, 3 * UNITS], bf16, tag="whh")

            x_tiles = {}

            def get_x_tile(jj, ooff, ff, step=2):
                key = (jj, ooff)
                if key not in x_tiles:
                    x_t = xpool.tile([P, CG, CH], bf16, tag="x", name="x_t")
                    bb = int(M_off[jj]) + ooff
                    for cp in range(0, CG, step):
                        nc.sync.dma_start(out=x_t[:, cp:cp + step, :ff],
                                          in_=xT[:, cp:cp + step, bb: bb + ff])
                    x_tiles[key] = x_t
                return x_tiles[key]

            def wih_slab(g, u0, u1):
                nc.sync.dma_start(
                    out=wih_t[:, :, g * UNITS + u0 * P: g * UNITS + u1 * P],
                    in_=wihT[:, :, g * UNITS + u0 * P: g * UNITS + u1 * P])

            # DMA emission order = consumption order: first x chunk, W_ih
            # slabs for the first unit pair, biases, the rest of W_ih, second
            # x chunk. W_hh and the presweep are emitted later. NOTE: this
            # exact order matters — reorderings have flipped the global Tile
            # schedule into a ~18% slower mode (vector backpressure on PSUM).
            ch0 = _chunks(m_j[0])
            get_x_tile(0, *ch0[0])
            for g in range(3):
                wih_slab(g, 0, 2)
            b_t = wpool.tile([P, UG, 4], f32, tag="bias")
            nc.sync.dma_start(out=b_t[:, :, :], in_=biases[:, :, :])
            for g in range(3):
                wih_slab(g, 2, 8)
            if len(ch0) > 1:
                get_x_tile(0, *ch0[1])

            def emit_whh():
                for g in range(3):
                    nc.sync.dma_start(
                        out=whh_t[:, :, g * UNITS:(g + 1) * UNITS],
                        in_=whhT[:, :, g * UNITS:(g + 1) * UNITS])

            gi_pre = (wpool.tile([P, 3 * UG, RN], dt.float16, tag="gi_pre",
                                name="gi_pre")
                      if RN > 0 else None)
            Copy = mybir.ActivationFunctionType.Identity  # Copy forbids AP bias

            def emit_presweep():
                # gi for all deep-pass rows in one efficient batched matmul.
                # Gate biases (b_sum_r, b_sum_z, b_ihn) are folded in here via
                # the activation bias, so pre-pass gates add gi_pre directly.
                with nc.named_scope("presweep"):
                    xp_t = xpool.tile([P, CG, RN], bf16, tag="x", name="xp_t")
                    for cp in range(0, CG, 2):
                        nc.sync.dma_start(out=xp_t[:, cp:cp + 2, :],
                                          in_=xT[:, cp:cp + 2, R0:N_pad])
                    for gu in range(3 * UG):
                        g, u = divmod(gu, UG)
                        ps_p = ppool.tile([P, CH], f32, tag="ps_gin",
                                          name="ps_pre")
                        for c in range(CG):
                            nc.tensor.matmul(
                                ps_p[:, :RN],
                                lhsT=wih_t[:, c, gu * P:(gu + 1) * P],
                                rhs=xp_t[:, c, :],
                                start=(c == 0), stop=(c == CG - 1))
                        nc.scalar.activation(gi_pre[:, gu, :], ps_p[:, :RN],
                                             Copy, bias=b_t[:, u, g:g + 1])

            if use_seed:
                emit_whh()  # pass 0 already needs W_hh

            # Fused-tail state: one PSUM bank per gate holds all UG unit
            # tiles (packed by m); elementwise runs as UG-wide ops; outputs
            # buffer in SBUF and ship in one DMA at the end.
            fused_state = {}

            f16 = dt.float16

            def emit_fused_chunk(j, h_cur, off, m, h_next, h_next_m=0):
                ob = fused_state["ob"]
                bb = fused_state["bb"]
                base = int(M_off[j])
                p0 = base + off - R0
                t0 = base + off - R5
                ps_z = ppool.tile([P, UG, FUSE_M], f32, tag="ps_z")
                ps_r = ppool.tile([P, UG, FUSE_M], f32, tag="ps_r")
                ps_n = ppool.tile([P, UG, FUSE_M], f32, tag="ps_ghn",
                                  name="ps_n")

                def gate_mms(gate, ps):
                    for u in range(UG):
                        for c in range(CG):
                            nc.tensor.matmul(
                                ps[:, u, :m],
                                lhsT=whh_t[:, c, gate * UNITS + u * P:
                                           gate * UNITS + (u + 1) * P],
                                rhs=h_cur[:, c, off:off + m],
                                start=(c == 0),
                                stop=(c == CG - 1),
                                skip_group_check=True)

                def gi3(g):
                    return gi_pre[:, g * UG:(g + 1) * UG, p0:p0 + m]

                def stile(tag):
                    return spool.tile([P, UG, FUSE_M], f16, tag=tag, name=tag,
                                      bufs=1)

                # z first: its whole chain hides under the r/ghn matmuls
                gate_mms(1, ps_z)
                z_sb = stile("fz")
                nc.vector.tensor_add(z_sb[:, :, :m], ps_z[:, :, :m], gi3(1))
                nc.scalar.activation(z_sb[:, :, :m], z_sb[:, :, :m], Sig)
                zh = stile("fzh")
                nc.vector.tensor_mul(zh[:, :, :m], z_sb[:, :, :m],
                                     h_cur[:, :, off:off + m])
                zm = stile("fzm")  # 1 - z
                nc.scalar.activation(zm[:, :, :m], z_sb[:, :, :m], Copy,
                                     bias=1.0, scale=-1.0)
                gate_mms(0, ps_r)
                r_sb = stile("fr")
                nc.vector.tensor_add(r_sb[:, :, :m], ps_r[:, :, :m], gi3(0))
                nc.scalar.activation(r_sb[:, :, :m], r_sb[:, :, :m], Sig)
                # n = tanh(gi_n + r*b_hhn + r*gh_n): r*b_hhn + gi_n computed
                # while the ghn matmuls stream, so the tail chain stays short
                rb = stile("ft")
                nc.vector.tensor_mul(rb[:, :, :m], r_sb[:, :, :m],
                                     bb[:, :, :m])
                grb = stile("fg")
                nc.vector.tensor_add(grb[:, :, :m], rb[:, :, :m], gi3(2))
                gate_mms(2, ps_n)
                g_sb = stile("ft")
                nc.vector.tensor_mul(g_sb[:, :, :m], ps_n[:, :, :m],
                                     r_sb[:, :, :m])
                n_sb = stile("fn")
                nc.vector.tensor_add(n_sb[:, :, :m], g_sb[:, :, :m],
                                     grb[:, :, :m])
                nc.scalar.activation(n_sb[:, :, :m], n_sb[:, :, :m], Tanh)
                a_sb = stile("fg")
                nc.vector.tensor_mul(a_sb[:, :, :m], zm[:, :, :m],
                                     n_sb[:, :, :m])
                if h_next is not None:
                    # split by c-halves: the next pass's first matmuls (c<4)
                    # unblock after the first half lands
                    nc.vector.tensor_add(h_next[:, 0:4, :],
                                         a_sb[:, 0:4, :h_next_m],
                                         zh[:, 0:4, :h_next_m])
                    nc.vector.tensor_add(h_next[:, 4:8, :],
                                         a_sb[:, 4:8, :h_next_m],
                                         zh[:, 4:8, :h_next_m])
                nc.vector.tensor_add(ob[:, :, t0:t0 + m], a_sb[:, :, :m],
                                     zh[:, :, :m])

            def emit_fused_pass(j, h_cur):
                if not fused_state:
                    ob = wpool.tile([P, UG, TAILN], f16, tag="outbuf")
                    bb = wpool.tile([P, UG, FUSE_M], f16, tag="bb")
                    nc.sync.dma_start(out=bb[:, :, :], in_=bhhn_bc[:, :, :])
                    fused_state.update(ob=ob, bb=bb)
                m = m_j[j]
                m_next = m_j[j + 1] if j + 1 < Lmax else 0
                h_next = (hpool.tile([P, CG, m_next], bf16, tag="hbuf",
                                     name=f"hbuf{j}") if m_next > 0 else None)
                # chunk 0 first: it alone carries h for the next pass, so its
                # chain (and h_next) completes while later chunks still stream
                for off in range(0, m, FUSE_M):
                    mc = min(FUSE_M, m - off)
                    emit_fused_chunk(j, h_cur, off, mc,
                                     h_next if off == 0 else None, m_next)
                if j == Lmax - 1:
                    # one fat-descriptor DMA for the whole tail (p-major)
                    nc.sync.dma_start(out=outTail[:, :, :],
                                      in_=fused_state["ob"][:, :, :])
                return h_next

            h_cur = None  # bf16 SBUF (P, CG, m_j[j]) input h for current pass
            for j in range(Lmax):
                if j == j_pre and gi_pre is not None:
                    emit_presweep()
                scope = nc.named_scope(f"pass{j:02d}")
                scope.__enter__()
                if j >= j_fuse:
                    h_cur = emit_fused_pass(j, h_cur)
                    scope.__exit__(None, None, None)
                    continue
                m = m_j[j]
                m_next = m_j[j + 1] if j + 1 < Lmax else 0
                has_h = (j > 0) or use_seed
                pre = j >= j_pre
                base = int(M_off[j])
                h_next = (hpool.tile([P, CG, m_next], bf16, tag="hbuf",
                                     name=f"hbuf{j}")
                          if m_next > 0 else None)

                for ci, (off, f) in enumerate(_chunks(m)):
                    if not pre:
                        x_t = get_x_tile(j, off, f)
                    if j == 0 and use_seed:
                        hs_t = xpool.tile([P, CG, CH], bf16, tag="hseed", name="hs_t", bufs=1)
                        for c in range(CG):
                            nc.sync.dma_start(
                                out=hs_t[:, c, :f],
                                in_=hseedT[c * P:(c + 1) * P, off: off + f])
                        h_in = lambda c: hs_t[:, c, :f]
                    elif has_h:
                        h_in = lambda c: h_cur[:, c, off: off + f]
                    else:
                        h_in = None
                    # presweep-relative row slice for this chunk
                    p0 = base + off - R0

                    def x_mms(ps, gate, stop_at_end):
                        for c in range(CG):
                            nc.tensor.matmul(
                                ps[:, :f],
                                lhsT=wih_t[:, c, gate * UNITS + u * P:
                                           gate * UNITS + (u + 1) * P],
                                rhs=x_t[:, c, :f],
                                start=(c == 0),
                                stop=(stop_at_end and c == CG - 1))

                    def h_mms(ps, gate, cs, do_start, do_stop):
                        cs = list(cs)
                        for c in cs:
                            nc.tensor.matmul(
                                ps[:, :f],
                                lhsT=whh_t[:, c, gate * UNITS + u * P:
                                           gate * UNITS + (u + 1) * P],
                                rhs=h_in(c),
                                start=(do_start and c == cs[0]),
                                stop=(do_stop and c == cs[-1]),
                                skip_group_check=True)

                    for u in range(UG):
                        ps_r = ppool.tile([P, CH], f32, tag="ps_r")
                        ps_z = ppool.tile([P, CH], f32, tag="ps_z")
                        if not pre:
                            ps_gin = ppool.tile([P, CH], f32, tag="ps_gin")
                        ps_ghn = (ppool.tile([P, CH], f32, tag="ps_ghn",
                                             name="ps_ghn")
                                  if has_h else None)

                        # For the first unit-tile of a chunk, defer every
                        # gate's c=7 h-matmul to the end: it waits on the
                        # previous pass's last h cast, and deferring lets the
                        # other 21+ matmuls run during that wait.
                        split = has_h and u == 0 and off == 0
                        early = range(CG - 1) if split else range(CG)
                        if not pre:
                            x_mms(ps_r, 0, stop_at_end=not has_h)
                            if has_h:
                                h_mms(ps_r, 0, early, False, not split)
                            x_mms(ps_z, 1, stop_at_end=not has_h)
                            if has_h:
                                h_mms(ps_z, 1, early, False, not split)
                            x_mms(ps_gin, 2, stop_at_end=True)
                            if has_h:
                                h_mms(ps_ghn, 2, early, True, not split)
                        else:
                            h_mms(ps_r, 0, early, True, not split)
                            h_mms(ps_z, 1, early, True, not split)
                            h_mms(ps_ghn, 2, early, True, not split)
                        if split:
                            h_mms(ps_r, 0, [CG - 1], False, True)
                            h_mms(ps_z, 1, [CG - 1], False, True)
                            h_mms(ps_ghn, 2, [CG - 1], False, True)

                        r_sb = spool.tile([P, CH], f32, tag="r")
                        z_sb = spool.tile([P, CH], f32, tag="z")
                        n_sb = spool.tile([P, CH], f32, tag="n")
                        h_sb = spool.tile([P, CH], f32, tag="r" if use_seed else "h",
                                          name="h_sb")
                        t2 = spool.tile([P, CH], f32, tag="t2")
                        if pre:
                            # biases already folded into gi_pre at presweep
                            nc.vector.tensor_add(r_sb[:, :f], ps_r[:, :f],
                                                 gi_pre[:, u, p0:p0 + f])
                            nc.scalar.activation(r_sb[:, :f], r_sb[:, :f], Sig)
                            nc.vector.tensor_add(z_sb[:, :f], ps_z[:, :f],
                                                 gi_pre[:, UG + u, p0:p0 + f])
                            nc.scalar.activation(z_sb[:, :f], z_sb[:, :f], Sig)
                            nc.vector.scalar_tensor_tensor(
                                t2[:, :f], ps_ghn[:, :f], b_t[:, u, 3:4],
                                r_sb[:, :f], op0=ADD, op1=MULT)
                            arg = spool.tile([P, CH], f32, tag="d", name="arg")
                            nc.vector.tensor_add(arg[:, :f], t2[:, :f],
                                                 gi_pre[:, 2 * UG + u, p0:p0 + f])
                            nc.scalar.activation(n_sb[:, :f], arg[:, :f], Tanh)
                        else:
                            nc.scalar.activation(r_sb[:, :f], ps_r[:, :f], Sig,
                                                 bias=b_t[:, u, 0:1])
                            nc.scalar.activation(z_sb[:, :f], ps_z[:, :f], Sig,
                                                 bias=b_t[:, u, 1:2])
                            if has_h:
                                # t2 = (ps_ghn + b_hhn) * r
                                nc.vector.scalar_tensor_tensor(
                                    t2[:, :f], ps_ghn[:, :f], b_t[:, u, 3:4],
                                    r_sb[:, :f], op0=ADD, op1=MULT)
                                arg = spool.tile([P, CH], f32, tag="d", name="arg")
                                nc.vector.tensor_add(arg[:, :f], t2[:, :f],
                                                     ps_gin[:, :f])
                                nc.scalar.activation(n_sb[:, :f], arg[:, :f],
                                                     Tanh, bias=b_t[:, u, 2:3])
                            else:
                                # t2 = r*b_hhn + ps_gin ; n = tanh(t2 + b_ihn)
                                nc.vector.scalar_tensor_tensor(
                                    t2[:, :f], r_sb[:, :f], b_t[:, u, 3:4],
                                    ps_gin[:, :f], op0=MULT, op1=ADD)
                                nc.scalar.activation(n_sb[:, :f], t2[:, :f],
                                                     Tanh, bias=b_t[:, u, 2:3])
                        if has_h:
                            # h = n + z*(h_prev - n)   (h_prev via bf16 tile)
                            d_sb = spool.tile([P, CH], f32, tag="d")
                            nc.vector.tensor_sub(d_sb[:, :f], h_in(u), n_sb[:, :f])
                            zd = spool.tile([P, CH], f32, tag="t2", name="zd")
                            nc.vector.tensor_mul(zd[:, :f], z_sb[:, :f], d_sb[:, :f])
                            nc.vector.tensor_add(h_sb[:, :f], n_sb[:, :f], zd[:, :f])
                        else:
                            # h = (1-z)*n = n - z*n
                            zd = spool.tile([P, CH], f32, tag="t2", name="zd")
                            nc.vector.tensor_mul(zd[:, :f], z_sb[:, :f], n_sb[:, :f])
                            nc.vector.tensor_sub(h_sb[:, :f], n_sb[:, :f], zd[:, :f])

                        nc.sync.dma_start(
                            out=outT[u * P:(u + 1) * P, base + off: base + off + f],
                            in_=h_sb[:, :f])
                        pf = min(m_next - off, f)
                        if pf > 0:
                            nc.vector.tensor_copy(h_next[:, u, off: off + pf],
                                                  h_sb[:, :pf])
                    if j == 0 and ci == 0 and not use_seed:
                        emit_whh()  # W_hh drains during pass-0 compute
                    if not pre and (j, off) in x_tiles:
                        del x_tiles[(j, off)]  # consumed; let the slot recycle
                h_cur = h_next
                scope.__exit__(None, None, None)
    nc.compile()
    return nc


# ------------------------------------------------------------------- kernel

def kernel(x, h0, reset, W_ih, W_hh, b_ih, b_hh):
    global LAST_EXEC_NS
    x = np.asarray(x, np.float32)
    h0 = np.asarray(h0, np.float32)
    reset_sb = np.asarray(reset).reshape(SEQ, B).astype(bool)
    W_ih = np.asarray(W_ih, np.float32)
    W_hh = np.asarray(W_hh, np.float32)
    b_ih = np.asarray(b_ih, np.float32)
    b_hh = np.asarray(b_hh, np.float32)

    h0_any = bool(np.any(h0))
    m_j, plans = _build_plan(reset_sb, h0_any)
    N_pad = sum(m_j)

    b_sum = b_ih + b_hh
    biases = np.stack([b_sum[:UNITS], b_sum[UNITS:2 * UNITS],
                       b_ih[2 * UNITS:], b_hh[2 * UNITS:]], axis=1)
    # p-major 3D layouts matching the device DMA access patterns
    biases3 = np.ascontiguousarray(
        biases.reshape(UG, P, 4).transpose(1, 0, 2), np.float32)
    wih3 = np.ascontiguousarray(
        W_ih.reshape(3 * UNITS, CG, P).transpose(2, 1, 0)).astype(np.float16)
    whh3 = np.ascontiguousarray(
        W_hh.reshape(3 * UNITS, CG, P).transpose(2, 1, 0)).astype(np.float16)
    bhhn = b_hh[2 * UNITS:].reshape(UG, P)  # (u, p)
    bhhn_bc = np.ascontiguousarray(
        np.broadcast_to(bhhn.T[:, :, None], (P, UG, FUSE_M))).astype(np.float16)

    j_pre = 1
    while j_pre < len(m_j) and sum(m_j[j_pre:]) > CH:
        j_pre += 1
    j_fuse = _pick_j_fuse(m_j, j_pre)
    has_fused = j_fuse < len(m_j)

    xf = x.reshape(SEQ * B, DIM)
    in_maps = []
    for c in range(NCORES):
        tok, seed_b = plans[c]
        real = tok >= 0
        xg = np.zeros((N_pad, DIM), np.float32)
        xg[real] = xf[tok[real]]
        m = {
            "xT": np.ascontiguousarray(
                xg.reshape(N_pad, CG, P).transpose(2, 1, 0)).astype(np.float16),
            "wihT": wih3, "whhT": whh3, "biases": biases3,
        }
        if has_fused:
            m["bhhn_bc"] = bhhn_bc
        if h0_any:
            hs = np.zeros((m_j[0], UNITS), np.float32)
            sreal = seed_b >= 0
            hs[sreal] = h0[seed_b[sreal]]
            m["hseedT"] = np.ascontiguousarray(hs.T).astype(np.float16)
        in_maps.append(m)

    nc = _build_nc(m_j, use_seed=h0_any, j_pre=j_pre)
    trace = os.environ.get("GRU_TRACE", "0") == "1"
    res = run_bass_kernel_spmd(nc, in_maps, list(range(NCORES)), trace=trace)
    global LAST_RES
    LAST_RES = res
    LAST_EXEC_NS = res.exec_time_ns

    R5 = int(sum(m_j[:j_fuse]))
    out = np.zeros((SEQ * B, UNITS), np.float32)
    for c in range(NCORES):
        tok, _ = plans[c]
        real = tok >= 0
        full = res.results[c]["outT"].T.copy()  # (N_pad, UNITS)
        if has_fused:
            tail = res.results[c]["outTail"]  # (P, UG, TAILN)
            full[R5:] = tail.transpose(2, 1, 0).reshape(
                N_pad - R5, UNITS).astype(np.float32)
        out[tok[real]] = full[real]
    return out.reshape(SEQ, B, UNITS)

